# revision 5
# baseline (speedup 1.0000x reference)
"""Trainium2 Bass kernel for nn_DenseExpert (soft-routed MoE dense layer).

Math:  out[b,u] = sum_e gate[b,e] * (x @ alpha[e])[b,u] + (gate @ beta)[b,u]
       B=32768, IN=1024, UNITS=1024, E=8.

Strategy (data-parallel over batch, 8 cores x 4096 rows):
  - All operands cast to fp16 on host (PE runs 1 cycle/row; fp32 PSUM accum).
  - Host pre-transposes x into k-tile-major layout and replicates the gate
    row across partitions, packed together per batch-tile so one DMA feeds
    the gate-scale multiply.
  - Device: per batch-tile of 128 rows, DVE computes the gate-scaled
    stationary tiles xg[e,k][i,b] = x^T[i,b]*g[b,e]; PE accumulates
    bias (g^T @ beta, K=8 matmul, opens the PSUM group) plus
    sum_{e,k} xg[e,k]^T @ alpha[e,k] into PSUM banks of 512 units;
    ACT drains PSUM to SBUF; DMA writes fp32 output.
  - alpha (16MB fp16) stays SBUF-resident for all 32 batch-tiles.
  - Startup ramp: the first RAMP batch-tiles are processed one 512-unit
    chunk at a time (uc=0 pass, then uc=1) so PE work starts once only
    half of alpha has landed; DMA runs at full bandwidth either way.
"""

import sys

sys.path.insert(0, "/opt/trn_rl_repo")

import numpy as np

B, IN, UNITS, E = 32768, 1024, 1024, 8
NCORES = 8
BL = B // NCORES          # rows per core
KT = IN // 128            # k-tiles (8)
UC = UNITS // 512         # 512-wide unit chunks (2)

_PROFILE = False          # set True (e.g. from test.py) to capture HW exec time
LAST_EXEC_NS = None
LAST_TRACE = None

_compiled = None


def _build(nbt):
    """Build + compile the per-core program. nbt = number of 128-row batch tiles."""
    import concourse.bacc as bacc
    import concourse.mybir as mybir
    import concourse.tile as tile
    from concourse.bass import ds, ts
    from contextlib import ExitStack

    f16 = mybir.dt.float16
    f32 = mybir.dt.float32

    ramp = min(8, nbt)    # batch-tiles processed per-unit-chunk during alpha load

    nc = bacc.Bacc("TRN2", target_bir_lowering=False, debug=False)

    xgsrc_t = nc.dram_tensor("xgsrc", [128, nbt, 2 * KT, 128], f16, kind="ExternalInput").ap()
    alpha_t = nc.dram_tensor("alpha", [128, UC, E, KT, 512], f16, kind="ExternalInput").ap()
    gtb_t = nc.dram_tensor("gtb", [E, nbt * 128 + UNITS], f16, kind="ExternalInput").ap()
    out_t = nc.dram_tensor("out", [nbt * 128, UNITS], f32, kind="ExternalOutput").ap()

    with tile.TileContext(nc) as tc, ExitStack() as ctx:
        const_pool = ctx.enter_context(tc.tile_pool(name="const", bufs=1))
        src_pool = ctx.enter_context(tc.tile_pool(name="src", bufs=3))
        xg_pool = ctx.enter_context(tc.tile_pool(name="xg", bufs=2))
        out_pool = ctx.enter_context(tc.tile_pool(name="ob", bufs=4))
        ps_pool = ctx.enter_context(tc.tile_pool(name="ps", bufs=4, space="PSUM"))

        gtb_sb = const_pool.tile([E, nbt * 128 + UNITS], f16, tag="gtb")
        nc.sync.dma_start(gtb_sb[:], gtb_t)
        alpha_sb = const_pool.tile([128, UC, E, KT, 512], f16, tag="alpha")
        for uc in range(UC):
            for e in range(E):
                nc.sync.dma_start(alpha_sb[:, uc, e], alpha_t[:, uc, e])

        def load_src(bt):
            src = src_pool.tile([128, 2 * KT, 128], f16, tag="src", name="src")
            nc.sync.dma_start(src[:], xgsrc_t[:, bt])
            return src

        def compute_xg(src):
            xg = xg_pool.tile([128, E, KT, 128], f16, tag="xg", name="xg")
            for e in range(E):
                nc.vector.tensor_tensor(
                    xg[:, e],
                    src[:, 0:KT, :],
                    src[:, KT + e : KT + e + 1, :].broadcast_to((128, KT, 128)),
                    op=mybir.AluOpType.mult,
                )
            return xg

        def group(bt, uc, xg):
            """One PSUM accumulation group: out[bt, uc*512:(uc+1)*512]."""
            ps = ps_pool.tile([128, 512], f32, tag=f"ps{uc}", name=f"ps{uc}")
            # bias opens the group: operands are SBUF-resident -> no fresh waits
            nc.tensor.matmul(
                ps[:],
                gtb_sb[:, ts(bt, 128)],
                gtb_sb[:, ds(nbt * 128 + uc * 512, 512)],
                start=True,
                stop=False,
            )
            for e in range(E):
                for k in range(KT):
                    nc.tensor.matmul(
                        ps[:],
                        xg[:, e, k],
                        alpha_sb[:, uc, e, k],
                        start=False,
                        stop=(e == E - 1 and k == KT - 1),
                    )
            ob = out_pool.tile([128, 512], f32, tag=f"ob{uc}", name=f"ob{uc}")
            nc.scalar.copy(ob[:], ps[:])
            nc.sync.dma_start(out_t[ts(bt, 128), ds(uc * 512, 512)], ob[:])

        # ramp: per-unit-chunk passes over the first `ramp` batch-tiles
        for uc in range(UC):
            for bt in range(ramp):
                xg = compute_xg(load_src(bt))
                group(bt, uc, xg)
        # steady state: both unit chunks per batch-tile
        for bt in range(ramp, nbt):
            xg = compute_xg(load_src(bt))
            for uc in range(UC):
                group(bt, uc, xg)

    nc.compile()
    return nc


def _prep_inputs(x, gate_perc, alpha, beta, ncores, nbt):
    """Host-side reshape/cast. Returns per-core input maps."""
    f16 = np.float16
    bl = nbt * 128

    x16 = np.ascontiguousarray(x, dtype=np.float32).astype(f16)
    # x part: [c, p, bt, k, bin] = x[c*bl + bt*128 + bin, k*128 + p]
    xpart = x16.reshape(ncores, nbt, 128, KT, 128).transpose(0, 4, 1, 3, 2)

    g16 = np.ascontiguousarray(gate_perc, dtype=np.float32).astype(f16)
    g5 = g16.reshape(ncores, nbt, 128, E)
    # g part: [c, p, bt, e, bin] = g[c, bt, bin, e] broadcast over p
    gpart = np.broadcast_to(g5[:, None], (ncores, 128, nbt, 128, E)).transpose(0, 1, 2, 4, 3)

    xgsrc = np.empty((ncores, 128, nbt, 2 * KT, 128), f16)
    xgsrc[:, :, :, 0:KT, :] = xpart
    xgsrc[:, :, :, KT:, :] = gpart

    alpha16 = np.ascontiguousarray(alpha, dtype=np.float32).astype(f16)
    # [p, uc, e, k, 512]
    asb = np.ascontiguousarray(alpha16.reshape(E, KT, 128, UC, 512).transpose(2, 3, 0, 1, 4))

    beta16 = np.ascontiguousarray(beta, dtype=np.float32).astype(f16)
    gtb = np.empty((ncores, E, bl + UNITS), f16)
    gtb[:, :, :bl] = g16.reshape(ncores, bl, E).transpose(0, 2, 1)
    gtb[:, :, bl:] = beta16[None]

    return [
        {"xgsrc": np.ascontiguousarray(xgsrc[c]), "alpha": asb, "gtb": np.ascontiguousarray(gtb[c])}
        for c in range(ncores)
    ]


def _profiled_spmd(nc, in_maps, core_ids):
    """run_bass_kernel_spmd wrapped in axon NTFF profiling; returns (res, exec_ns, trace)."""
    import glob
    import tempfile

    import concourse.bass_utils as bu
    from concourse._compat import FishPath
    from trn_agent_boot.trn_boot import _ntff_profile_via_ctypes

    import gauge.profiler

    hook = _ntff_profile_via_ctypes("/opt/axon/libaxon_pjrt.so")
    outdir = tempfile.mkdtemp(prefix="ntff_")
    with hook(outdir, [0]):
        res = bu.run_bass_kernel_spmd(nc, in_maps, core_ids=core_ids, trace=False)
    ntffs = sorted(glob.glob(outdir + "/*.ntff"))
    print(f"profile: ntff files in {outdir}: {[f.split('/')[-1] for f in ntffs]}")
    if not ntffs:
        return res, None, None
    profile = gauge.profiler.Profile(
        profile_path=FishPath(outdir),
        kernel_dev_mode=True,
        profile_on_exit=False,
        bass_kernel=nc.m,
        offline_processing=True,
        fname="*_body*",
        metadata={},
    )
    model_indices = sorted({n.model_index for n in profile.find_ntffs()})
    perf = bu._process_ntff_profile(
        profile, outdir, nc, core_ids, list(model_indices), False, {}, False
    )
    trace_path = perf.insts_and_trace_path[1] if perf.insts_and_trace_path else None
    return res, perf.exec_time_ns, trace_path


def run(x, gate_perc, alpha, beta, profile=False):
    global _compiled, LAST_EXEC_NS, LAST_TRACE
    from concourse.bass_utils import run_bass_kernel_spmd

    nbt = BL // 128
    if _compiled is None:
        _compiled = _build(nbt)
    nc = _compiled

    in_maps = _prep_inputs(
        np.asarray(x), np.asarray(gate_perc), np.asarray(alpha), np.asarray(beta), NCORES, nbt
    )
    core_ids = list(range(NCORES))
    if profile:
        res, LAST_EXEC_NS, LAST_TRACE = _profiled_spmd(nc, in_maps, core_ids)
    else:
        res = run_bass_kernel_spmd(nc, in_maps, core_ids=core_ids, trace=False)
    out = np.concatenate([res.results[c]["out"] for c in range(NCORES)], axis=0)
    return out.astype(np.float32, copy=False)


def kernel(x, gate_perc, alpha, beta):
    return run(x, gate_perc, alpha, beta, profile=_PROFILE)


# revision 8
# speedup vs baseline: 1.0522x; 1.0522x over previous
"""Trainium2 Bass kernel for nn_DenseExpert (soft-routed MoE dense layer).

Math:  out[b,u] = sum_e gate[b,e] * (x @ alpha[e])[b,u] + (gate @ beta)[b,u]
       B=32768, IN=1024, UNITS=1024, E=8.

Strategy (data-parallel over batch, 8 cores x 4096 rows):
  - All operands cast to fp16 on host (PE runs 1 cycle/row; fp32 PSUM accum).
  - Host pre-transposes x into k-tile-major layout and replicates the gate
    row across partitions, packed together per batch-tile so one DMA feeds
    the gate-scale multiply.
  - Device: per batch-tile of 128 rows, DVE computes the gate-scaled
    stationary tiles xg[e,k][i,b] = x^T[i,b]*g[b,e]; PE accumulates
    bias (g^T @ beta, K=8 matmul, opens the PSUM group) plus
    sum_{e,k} xg[e,k]^T @ alpha[e,k] into PSUM banks of 512 units;
    ACT drains PSUM to SBUF; DMA writes fp32 output.
  - alpha (16MB fp16) stays SBUF-resident for all 32 batch-tiles.
  - Startup ramp: the first RAMP batch-tiles are processed one 512-unit
    chunk at a time (uc=0 pass, then uc=1) so PE work starts once only
    half of alpha has landed; DMA runs at full bandwidth either way.
"""

import sys

sys.path.insert(0, "/opt/trn_rl_repo")

import numpy as np

B, IN, UNITS, E = 32768, 1024, 1024, 8
NCORES = 8
BL = B // NCORES          # rows per core
KT = IN // 128            # k-tiles (8)
UC = UNITS // 512         # 512-wide unit chunks (2)

_PROFILE = False          # set True (e.g. from test.py) to capture HW exec time
LAST_EXEC_NS = None
LAST_TRACE = None

_compiled = None


def _build(nbt):
    """Build + compile the per-core program. nbt = number of 128-row batch tiles."""
    import concourse.bacc as bacc
    import concourse.mybir as mybir
    import concourse.tile as tile
    from concourse.bass import ds, ts
    from contextlib import ExitStack

    f16 = mybir.dt.float16
    f32 = mybir.dt.float32

    ramp = min(8, nbt)    # batch-tiles processed per-unit-chunk during alpha load

    nc = bacc.Bacc("TRN2", target_bir_lowering=False, debug=False)

    xgsrc_t = nc.dram_tensor("xgsrc", [128, nbt, 2 * KT, 128], f16, kind="ExternalInput").ap()
    alpha_t = nc.dram_tensor("alpha", [128, UC, E, KT, 512], f16, kind="ExternalInput").ap()
    # gate^T and beta zero-padded to 128 contraction rows so the bias matmul
    # uses the same full-K LDWEIGHTS path as the main stream (no row-group
    # conflict bubble at group boundaries)
    gtb_t = nc.dram_tensor("gtb", [128, nbt * 128 + UNITS], f16, kind="ExternalInput").ap()
    out_t = nc.dram_tensor("out", [nbt * 128, UNITS], f32, kind="ExternalOutput").ap()

    with tile.TileContext(nc) as tc, ExitStack() as ctx:
        const_pool = ctx.enter_context(tc.tile_pool(name="const", bufs=1))
        src_pool = ctx.enter_context(tc.tile_pool(name="src", bufs=4))
        xg_pool = ctx.enter_context(tc.tile_pool(name="xg", bufs=2))
        out_pool = ctx.enter_context(tc.tile_pool(name="ob", bufs=4))
        ps_pool = ctx.enter_context(tc.tile_pool(name="ps", bufs=4, space="PSUM"))

        # src/gtb/out ride the Activation HW-DGE queue; alpha rides the SP
        # queue. Queues drain in FIFO order, so this keeps the first src
        # tiles from queueing behind 16MB of alpha.
        gtb_sb = const_pool.tile([128, nbt * 128 + UNITS], f16, tag="gtb")
        nc.scalar.dma_start(gtb_sb[:], gtb_t)

        def load_src(bt):
            src = src_pool.tile([128, 2 * KT, 128], f16, tag="src", name="src")
            nc.scalar.dma_start(src[:], xgsrc_t[:, bt])
            return src

        early_src = [load_src(bt) for bt in range(min(3, nbt))]

        alpha_sb = const_pool.tile([128, UC, E, KT, 512], f16, tag="alpha")
        for uc in range(UC):
            for e in range(E):
                nc.sync.dma_start(alpha_sb[:, uc, e], alpha_t[:, uc, e])

        def compute_xg(src):
            xg = xg_pool.tile([128, E, KT, 128], f16, tag="xg", name="xg")
            for e in range(E):
                nc.vector.tensor_tensor(
                    xg[:, e],
                    src[:, 0:KT, :],
                    src[:, KT + e : KT + e + 1, :].broadcast_to((128, KT, 128)),
                    op=mybir.AluOpType.mult,
                )
            return xg

        def group(bt, uc, xg):
            """One PSUM accumulation group: out[bt, uc*512:(uc+1)*512]."""
            ps = ps_pool.tile([128, 512], f32, tag=f"ps{uc}", name=f"ps{uc}")
            # bias opens the group: operands are SBUF-resident -> no fresh waits
            nc.tensor.matmul(
                ps[:],
                gtb_sb[:, ts(bt, 128)],
                gtb_sb[:, ds(nbt * 128 + uc * 512, 512)],
                start=True,
                stop=False,
            )
            for e in range(E):
                for k in range(KT):
                    nc.tensor.matmul(
                        ps[:],
                        xg[:, e, k],
                        alpha_sb[:, uc, e, k],
                        start=False,
                        stop=(e == E - 1 and k == KT - 1),
                    )
            ob = out_pool.tile([128, 512], f32, tag=f"ob{uc}", name=f"ob{uc}")
            nc.scalar.copy(ob[:], ps[:])
            nc.scalar.dma_start(out_t[ts(bt, 128), ds(uc * 512, 512)], ob[:])

        def get_src(bt):
            if bt < len(early_src):
                return early_src[bt]
            return load_src(bt)

        # ramp: per-unit-chunk passes over the first `ramp` batch-tiles
        for uc in range(UC):
            for bt in range(ramp):
                xg = compute_xg(get_src(bt) if uc == 0 else load_src(bt))
                group(bt, uc, xg)
        # steady state: both unit chunks per batch-tile
        for bt in range(ramp, nbt):
            xg = compute_xg(load_src(bt))
            for uc in range(UC):
                group(bt, uc, xg)

    nc.compile()
    return nc


def _prep_inputs(x, gate_perc, alpha, beta, ncores, nbt):
    """Host-side reshape/cast. Returns per-core input maps."""
    f16 = np.float16
    bl = nbt * 128

    x16 = np.ascontiguousarray(x, dtype=np.float32).astype(f16)
    # x part: [c, p, bt, k, bin] = x[c*bl + bt*128 + bin, k*128 + p]
    xpart = x16.reshape(ncores, nbt, 128, KT, 128).transpose(0, 4, 1, 3, 2)

    g16 = np.ascontiguousarray(gate_perc, dtype=np.float32).astype(f16)
    g5 = g16.reshape(ncores, nbt, 128, E)
    # g part: [c, p, bt, e, bin] = g[c, bt, bin, e] broadcast over p
    gpart = np.broadcast_to(g5[:, None], (ncores, 128, nbt, 128, E)).transpose(0, 1, 2, 4, 3)

    xgsrc = np.empty((ncores, 128, nbt, 2 * KT, 128), f16)
    xgsrc[:, :, :, 0:KT, :] = xpart
    xgsrc[:, :, :, KT:, :] = gpart

    alpha16 = np.ascontiguousarray(alpha, dtype=np.float32).astype(f16)
    # [p, uc, e, k, 512]
    asb = np.ascontiguousarray(alpha16.reshape(E, KT, 128, UC, 512).transpose(2, 3, 0, 1, 4))

    beta16 = np.ascontiguousarray(beta, dtype=np.float32).astype(f16)
    gtb = np.zeros((ncores, 128, bl + UNITS), f16)
    gtb[:, :E, :bl] = g16.reshape(ncores, bl, E).transpose(0, 2, 1)
    gtb[:, :E, bl:] = beta16[None]

    return [
        {"xgsrc": np.ascontiguousarray(xgsrc[c]), "alpha": asb, "gtb": np.ascontiguousarray(gtb[c])}
        for c in range(ncores)
    ]


def _profiled_spmd(nc, in_maps, core_ids):
    """run_bass_kernel_spmd wrapped in axon NTFF profiling; returns (res, exec_ns, trace)."""
    import glob
    import tempfile

    import concourse.bass_utils as bu
    from concourse._compat import FishPath
    from trn_agent_boot.trn_boot import _ntff_profile_via_ctypes

    import gauge.profiler

    hook = _ntff_profile_via_ctypes("/opt/axon/libaxon_pjrt.so")
    outdir = tempfile.mkdtemp(prefix="ntff_")
    with hook(outdir, [0]):
        res = bu.run_bass_kernel_spmd(nc, in_maps, core_ids=core_ids, trace=False)
    ntffs = sorted(glob.glob(outdir + "/*.ntff"))
    print(f"profile: ntff files in {outdir}: {[f.split('/')[-1] for f in ntffs]}")
    if not ntffs:
        return res, None, None
    profile = gauge.profiler.Profile(
        profile_path=FishPath(outdir),
        kernel_dev_mode=True,
        profile_on_exit=False,
        bass_kernel=nc.m,
        offline_processing=True,
        fname="*_body*",
        metadata={},
    )
    model_indices = sorted({n.model_index for n in profile.find_ntffs()})
    perf = bu._process_ntff_profile(
        profile, outdir, nc, core_ids, list(model_indices), False, {}, False
    )
    trace_path = perf.insts_and_trace_path[1] if perf.insts_and_trace_path else None
    return res, perf.exec_time_ns, trace_path


def run(x, gate_perc, alpha, beta, profile=False):
    global _compiled, LAST_EXEC_NS, LAST_TRACE
    from concourse.bass_utils import run_bass_kernel_spmd

    nbt = BL // 128
    if _compiled is None:
        _compiled = _build(nbt)
    nc = _compiled

    in_maps = _prep_inputs(
        np.asarray(x), np.asarray(gate_perc), np.asarray(alpha), np.asarray(beta), NCORES, nbt
    )
    core_ids = list(range(NCORES))
    if profile:
        res, LAST_EXEC_NS, LAST_TRACE = _profiled_spmd(nc, in_maps, core_ids)
    else:
        res = run_bass_kernel_spmd(nc, in_maps, core_ids=core_ids, trace=False)
    out = np.concatenate([res.results[c]["out"] for c in range(NCORES)], axis=0)
    return out.astype(np.float32, copy=False)


def kernel(x, gate_perc, alpha, beta):
    return run(x, gate_perc, alpha, beta, profile=_PROFILE)
